# revision 44
# baseline (speedup 1.0000x reference)
"""Trainium2 Bass kernel for nn_MultiHeadedSelfAttention_5179730559275.

Reference math (per batch b):
  q = wq @ x + bq ; k = wk @ x + bk ; v = wv @ x + bv        (1x1 conv, C=256 -> O=256)
  per o-channel (o = head*32 + d), with Q_o,K_o,V_o = 64x64 images [H,W]:
    S_o = Q_o @ K_o^T / sqrt(32); P_o = softmax(S_o, axis=-1); ctx_o = P_o @ V_o

Sharding: data-parallel over batch, 2 batches per core on 8 cores.

v2 design notes (vs v1): DMA instruction count collapsed (each DMA
instruction costs ~625ns on the shared HWDGE ring):
  - per-h [128x128] xbar transposes -> ONE batched dma_start_transpose per
    tensor per batch (out[a,b,c] = in[c, b*128+a] gives the per-128-block
    transpose directly).
  - per-jg output stores -> per-batch accumulation tile + 2 DMAs.
  - psum->SBUF projection copies batched over 2 psum banks [128,1024].
  - bv folded into the V projection (softmax rows sum to 1), so the final
    normalize is a pure scale; done as ONE broadcast tensor_tensor per
    4-image psum group on the otherwise-idle GPSIMD engine.

Per-core pipeline (per batch):
  1. fp16 x tiles [c, pix] -> PE projections (lhsT = w^T fp16 stationary,
     rhs = x fp16 moving, N=512, 2 psum banks per copy group)
  2. psum->SBUF copies add bias, cast fp16, write interleaved layouts
     pairing o with o+128 (om = o chunk):
       q16/k16: [j, h*128 + om*64 + w]   (j = o mod 128)
       v16:     [j, w*128 + om*64 + g]
  3. one batched xbar-transpose per tensor gives matmul-ready layouts:
       qS/kS: [om*64 + w, h, j]  (per-o transposed images, o-pair stacked)
       vS:    [om*64 + g, w, j]  (natural images + ones column for Z)
  4. Attention per pair j: quadrant matmuls (K=64 at partition bases 0/64):
       S^T psum [om*64+g, h] ; exp (ACT, bias -2) -> eS fp16
       ctx psum [om*64+h, 0:64]=E^T.T@V, col 64 = Z (ones column)
     normalize: ctx = psum * (1/Z) via broadcast multiply into ocF,
     then 2 output DMAs per batch.
"""

import numpy as np

import concourse.bass as bass
import concourse.bacc as bacc
import concourse.tile as tile
from concourse import mybir
from concourse import bass2jax

NCORES = 8
B, C, H, W = 16, 256, 64, 64
O = 256
PIX = H * W
BL = B // NCORES  # batches per core
SCALE = 1.0 / float(np.sqrt(32.0))
EXP_BIAS = -2.0  # softmax-invariant shift keeping exp() well inside fp16 range

FP32 = mybir.dt.float32
FP16 = mybir.dt.float16


def build_kernel(nc: bass.Bass):
    x_in = nc.declare_dram_parameter("x", [BL, C, PIX], FP16, isOutput=False)
    wT_in = nc.declare_dram_parameter("wT", [3, C, O], FP16, isOutput=False)
    bias_in = nc.declare_dram_parameter("bias", [3, O], FP32, isOutput=False)
    out = nc.declare_dram_parameter("out", [BL, O, PIX], FP16, isOutput=True)

    with tile.TileContext(nc) as tc:
        with (
            tc.tile_pool(name="singles", bufs=1) as singles,
            tc.tile_pool(name="xin", bufs=2) as xpool,
            tc.tile_pool(name="p16", bufs=1) as p16pool,
            tc.tile_pool(name="tsp", bufs=2) as tpool,
            tc.tile_pool(name="ocf", bufs=3) as ocpool,
            tc.tile_pool(name="small", bufs=3) as small,
            tc.tile_pool(name="psA", bufs=2, space="PSUM") as psA,
            tc.tile_pool(name="psS", bufs=2, space="PSUM") as psS,
            tc.tile_pool(name="psC", bufs=2, space="PSUM") as psC,
        ):
            # ---- constants loaded once (SWDGE queue; keeps the SP/HWDGE
            # queue free for the ordering-sensitive transposes) ----
            w_sb = singles.tile([128, 3, 2, O], FP16)  # [c', proj, cc, o]
            nc.gpsimd.dma_start(
                out=w_sb,
                in_=wT_in.rearrange("t (cc c) o -> c t cc o", cc=2),
            )
            bias_sb = singles.tile([128, 3, 2], FP32)  # [o', proj, oc]
            nc.gpsimd.dma_start(
                out=bias_sb,
                in_=bias_in.rearrange("t (oc o) -> o t oc", oc=2),
            )
            expb_sb = singles.tile([128, 1], FP32)
            nc.vector.memset(expb_sb, EXP_BIAS)

            tensors = {}
            JG = 8
            PG = 4
            PROJ_ORDER = (2, 0, 1)  # v first (ctx gating), k last

            def front_gen(b):
                """Projection front for batch b. Yields after each psum-tile
                group so the caller can interleave attention stages of the
                previous batch into every engine queue."""
                xt = xpool.tile([128, 2, PIX], FP16, tag="xsb")
                xdram = x_in[b].rearrange("(cc c) pix -> c cc pix", cc=2)
                # split the load so the first matmul group starts early; the
                # first batch gets a tiny leading piece to cut startup latency
                cuts = (
                    [0, 512, 1024, 2048, 3072, PIX]
                    if b == 0
                    else [0, 1024, 2048, 3072, PIX]
                )
                for lo, hi in zip(cuts, cuts[1:]):
                    nc.gpsimd.dma_start(out=xt[:, :, lo:hi], in_=xdram[:, :, lo:hi])
                xsb = [xt[:, 0, :], xt[:, 1, :]]

                q16 = p16pool.tile([128, H, 2, W], FP16, tag="q16")  # [j, h, om, w]
                k16 = p16pool.tile([128, H, 2, W], FP16, tag="k16")
                v16 = p16pool.tile([128, W, 2, H], FP16, tag="v16")  # [j, w, om, g]

                qS = tpool.tile([128, H, 128], FP16, tag="qS")  # [om*64+w, h, j]
                kS = tpool.tile([128, H, 128], FP16, tag="kS")
                vS = tpool.tile([128, W + 1, 128], FP16, tag="vS")
                nc.gpsimd.memset(vS[:, W, :], 1.0)
                tensors[b] = (qS, kS, vS)

                # per projection: 16 single-bank psum tiles [128,512], each
                # filled by 2 cc-accumulation matmuls, then ONE batched copy
                # (bias add + fp16 cast). 4-deep psum rotation keeps PE ahead
                # of the copy drain; copies split 10 ACT / 6 DVE per proj so
                # neither engine paces a phase and totals stay balanced.
                for proj in PROJ_ORDER:
                    for oc in range(2):
                        for grp in range(8):
                            ps = psA.tile(
                                [128, 512], FP32, tag="ps_proj", bufs=4, name="ps"
                            )
                            pix0 = oc * 0 + grp * 512
                            for cc in range(2):
                                nc.tensor.matmul(
                                    ps,
                                    lhsT=w_sb[:, proj, cc, oc * 128 : (oc + 1) * 128],
                                    rhs=xsb[cc][:, pix0 : pix0 + 512],
                                    start=(cc == 0),
                                    stop=(cc == 1),
                                )
                            bias_ap = bias_sb[:, proj, oc : oc + 1]
                            h0 = grp * 8  # 8 image rows per copy group
                            if proj != 2:  # q/k: [p, h-slice, oc, w]
                                outp = (q16, k16)[proj][:, h0 : h0 + 8, oc, :]
                                inp = ps.rearrange("p (h w) -> p h w", w=W)
                            else:  # v transposed view [p, w, g-blk], bias=bv
                                outp = v16[:, :, oc, h0 : h0 + 8]
                                inp = ps.rearrange("p (g w) -> p w g", w=W)
                            if (oc * 8 + grp) % 8 not in (2, 5, 7):
                                nc.scalar.activation(
                                    out=outp,
                                    in_=inp,
                                    func=mybir.ActivationFunctionType.Identity,
                                    bias=bias_ap,
                                    scale=1.0,
                                )
                            else:
                                nc.vector.tensor_scalar_add(
                                    out=outp, in0=inp, scalar1=bias_ap
                                )
                            yield
                    # batched transpose of the finished tensor, split along j
                    # (the source partition dim). The last projection (k)
                    # gates the next attention start, so it goes in eighths
                    # to minimize first-transfer latency; others in halves.
                    src = (q16, k16, v16)[proj]
                    dst = (qS, kS, vS)[proj]
                    npc = 4 if proj == PROJ_ORDER[-1] else 2
                    jw = 128 // npc
                    for jp in range(npc):
                        nc.sync.dma_start_transpose(
                            out=dst[:, 0:64, jp * jw : (jp + 1) * jw]
                            if proj == 2
                            else dst[:, :, jp * jw : (jp + 1) * jw],
                            in_=src[jp * jw : (jp + 1) * jw].rearrange(
                                "p a om b -> p (a om b)"
                            ),
                        )

            def attn_gen(b, deferred_outs):
                """Attention for batch b as 17 stages. Stage r emits the S
                matmuls + exp for group r and the ctx/normalize for group r-1
                (one stage behind so PE never head-blocks on exp latency).
                Output DMAs are deferred for non-final batches so they queue
                on SP after the next batch's transposes, not before."""
                qS, kS, vS = tensors[b]
                state = {"ocq": None, "jq0": 0}

                def emit_scores(jg):
                    sp8f = psS.tile([128, 512], FP32, tag="sp8", name="sp8f")
                    sp8 = sp8f.rearrange("p (i h) -> p i h", h=H)
                    for i in range(JG):
                        j = jg + i
                        for om in range(2):
                            pr = slice(om * 64, om * 64 + 64)
                            nc.tensor.matmul(
                                sp8[pr, i, :],
                                lhsT=kS[pr, :, j],
                                rhs=qS[pr, :, j],
                                start=True,
                                stop=True,
                            )
                    eS8 = small.tile([128, JG, H], FP16, tag="eS8", name="eS8")
                    nc.scalar.activation(
                        out=eS8,
                        in_=sp8,
                        func=mybir.ActivationFunctionType.Exp,
                        bias=expb_sb,
                        scale=1.0,
                    )
                    return eS8

                def emit_ctx(jg, eS8):
                    if jg % 32 == 0:
                        state["ocq"] = ocpool.tile(
                            [128, 32, W], FP16, tag="ocq", bufs=5, name="ocq"
                        )
                        state["jq0"] = jg
                    ocq, jq0 = state["ocq"], state["jq0"]
                    for sg in range(jg, jg + JG, PG):
                        # last batch: borrow the (idle) projection psum banks
                        # for extra ctx rotation depth — psC alone lockstepping
                        # at 2 buffers paces the whole solo-attention chain
                        if b == BL - 1:
                            cp4f = (psC, psA)[(sg // PG) % 2].tile(
                                [128, 512],
                                FP32,
                                tag="cp4" if (sg // PG) % 2 == 0 else "ps_proj",
                                bufs=2 if (sg // PG) % 2 == 0 else 4,
                                name="cp4f",
                            )
                        else:
                            cp4f = psC.tile(
                                [128, 512], FP32, tag="cp4", name="cp4f"
                            )
                        cp4 = cp4f[:, 0 : PG * (W + 1)].rearrange(
                            "p (i c) -> p i c", c=W + 1
                        )
                        for i in range(PG):
                            j = sg + i
                            for om in range(2):
                                pr = slice(om * 64, om * 64 + 64)
                                nc.tensor.matmul(
                                    cp4[pr, i, :],
                                    lhsT=eS8[pr, j - jg, :],
                                    rhs=vS[pr, :, j],
                                    start=True,
                                    stop=True,
                                )
                        # ctx = psum * (1/Z); bv already folded into V
                        rz4 = small.tile([128, PG], FP32, tag="rz4", name="rz4")
                        nc.vector.reciprocal(out=rz4, in_=cp4[:, :, W])
                        rz_ap = rz4[:, :]
                        rz_bcast = bass.AP(
                            tensor=rz_ap.tensor,
                            offset=rz_ap.offset,
                            ap=[[rz_ap.ap[0][0], 128], [1, PG], [0, W]],
                        )
                        nc.vector.tensor_tensor(
                            out=ocq[:, sg - jq0 : sg - jq0 + PG, :],
                            in0=cp4[:, :, 0:W],
                            in1=rz_bcast,
                            op=mybir.AluOpType.mult,
                        )
                    if (jg + JG) % 32 == 0:
                        out_ap = out[b, :, :]

                        def emit_out(ocq=ocq, jq0=jq0):
                            for om in range(2):
                                dst = bass.AP(
                                    tensor=out_ap.tensor,
                                    offset=out_ap.offset + (om * 128 + jq0) * PIX,
                                    ap=[[W, 64], [PIX, 32], [1, W]],  # (h, j, w)
                                )
                                nc.sync.dma_start(
                                    out=dst,
                                    in_=ocq[om * 64 : om * 64 + 64, :, :],
                                )

                        if deferred_outs is None:
                            emit_out()  # final batch: SP queue is free now
                        else:
                            deferred_outs.append(emit_out)

                pending = None
                for jg in range(0, 128, JG):
                    eS8 = emit_scores(jg)
                    if pending is not None:
                        emit_ctx(*pending)
                    pending = (jg, eS8)
                    yield
                emit_ctx(*pending)
                yield

            # Interleaved emission: engine queues are FIFO, so attention of
            # batch b is pumped stage-by-stage between the projection psum
            # tiles of batch b+1 — PE fills attention chain latency with
            # projection matmuls and neither stream head-blocks the other.
            # Pumping starts a few tiles in so the S matmuls' transpose
            # inputs are ready before they reach the PE queue head.
            # PE warmup: a few tiny junk matmuls as soon as the weights land
            # so the clock-ramp is warm when the first x piece arrives
            wps = psA.tile([128, 512], FP32, tag="ps_proj", bufs=4, name="wps")
            wrhs = w_sb.rearrange("p t cc o -> p (t cc o)")[:, 0:128]
            for r in range(6):
                nc.tensor.matmul(
                    wps[:, 0:128],
                    lhsT=w_sb[:, 0, 0, 0:128],
                    rhs=wrhs,
                    start=(r == 0),
                    stop=(r == 5),
                )

            PUMP_START = 12  # tiles before the first pumped attention stage
            ag = None
            pending_outs = []
            for b in range(BL):
                for i, _ in enumerate(front_gen(b)):
                    if ag is not None and i >= PUMP_START and (i - PUMP_START) % 2 == 0:
                        next(ag, None)
                # previous batch's output stores, queued after this batch's
                # transposes so they don't head-block them on SP
                for f in pending_outs:
                    f()
                pending_outs = []
                ag = attn_gen(b, pending_outs if b < BL - 1 else None)
            for _ in ag:
                pass
    return nc


_NC_CACHE = {}


def get_nc():
    if "nc" not in _NC_CACHE:
        nc = bacc.Bacc(None, target_bir_lowering=False)
        build_kernel(nc)
        nc.finalize()
        _NC_CACHE["nc"] = nc
    return _NC_CACHE["nc"]


def prep_in_maps(x, wq, bq, wk, bk, wv, bv):
    wT = np.stack(
        [
            np.ascontiguousarray((wq * SCALE).T),
            np.ascontiguousarray(wk.T),
            np.ascontiguousarray(wv.T),
        ]
    ).astype(np.float16)
    biases = np.stack([bq * SCALE, bk, bv]).astype(np.float32)
    xs = np.ascontiguousarray(x.reshape(NCORES, BL, C, PIX)).astype(np.float16)
    return [{"x": xs[i], "wT": wT, "bias": biases} for i in range(NCORES)]


def kernel(x, wq, bq, wk, bk, wv, bv):
    nc = get_nc()
    in_maps = prep_in_maps(x, wq, bq, wk, bk, wv, bv)
    results = bass2jax.run_bass_via_pjrt(nc, in_maps, n_cores=NCORES)
    outs = [np.asarray(r["out"]).reshape(BL, O, H, W) for r in results]
    return np.concatenate(outs, axis=0).astype(np.float32)
